# revision 5
# baseline (speedup 1.0000x reference)
"""Single-head causal attention on 8 TRN2 NeuronCores.

Sharding: 8 cores = 4 batches x 2 folded query-halves.
  fold 0 owns global q rows [0:512) + [1536:2048)
  fold 1 owns global q rows [512:1536)
Each core loads tokens[b] with its rows permuted so its OWN q rows come
first, projects K/V for all 2048 rows (duplicated across the pair; cheap)
and Q for its own 1024 rows, computes S^T = K @ Q^T tiles, exp (no max
subtraction needed: |scores| < ~1), causal mask via per-core 0/1 tables
(multiplied after exp), AV^T accumulated with a ones-augmented V to get
row sums, PE-transposes the output back to [q, h] and divides by the sum.

The graph is identical on all 8 cores (SPMD); all per-core asymmetry
(fold, causal offsets) lives in the input data (permutation + mask
tables computed on host).
"""

import numpy as np
import ml_dtypes

BF16 = ml_dtypes.bfloat16

E = 1024      # embed dim
S = 2048      # sequence length
H = 64        # head dim
B = 4         # batch
NC = 8        # cores
NT = S // 128     # 16 seq tiles of 128
CH = 512          # q-chunk width
SLOTS_A = 8       # k-tile slots for first q-chunk
SLOTS_B = 16      # k-tile slots for second q-chunk
PHI_A = [0, 1, 2, 3, 8, 9, 10, 11]   # slot -> physical k-tile, chunk A
PHI_B = list(range(16))              # slot -> physical k-tile, chunk B
TBL_W = 896       # mask table width: 384 + 512

_COMPILED = None


def _perm(fold: int) -> np.ndarray:
    """Global row order of a core's shard: own 1024 q rows first."""
    lo = np.arange(0, 512)
    mid = np.arange(512, 1536)
    hi = np.arange(1536, 2048)
    if fold == 0:
        return np.concatenate([lo, hi, mid])
    return np.concatenate([mid, lo, hi])


def _gmap(fold: int) -> list:
    """G[s] = global k-tile index of physical tile s in the permuted shard."""
    p = _perm(fold)
    return [int(p[128 * t] // 128) for t in range(NT)]


def _mask_tables(fold: int) -> np.ndarray:
    """[6, 128, TBL_W] bf16: (chunk A: 2 blocks of 4 slots, chunk B: 4 blocks).

    Table (c, beta) covers slots 4*beta..4*beta+3 of chunk c; slot s slices
    the table at offset 384 - 128*(s % 4), width 512:
        T[p, g] = 1.0 if p <= g + C  with  C = Q0_c - 128*G(phi_c(4*beta)) - 384
    """
    G = _gmap(fold)
    q0 = {0: (0, 1536), 1: (512, 1024)}[fold]  # global Q0 of (chunk A, chunk B)
    tables = []
    for ci, (phi, nblk) in enumerate([(PHI_A, 2), (PHI_B, 4)]):
        for beta in range(nblk):
            C = q0[ci] - 128 * G[phi[4 * beta]] - 384
            g = np.arange(TBL_W)[None, :]
            p = np.arange(128)[:, None]
            tables.append((p <= g + C).astype(np.float32))
    return np.stack(tables).astype(BF16)


def _build():
    from concourse import bacc, tile, mybir
    from contextlib import ExitStack

    f32 = mybir.dt.float32
    bf16 = mybir.dt.bfloat16

    nc = bacc.Bacc()
    d_tok = nc.declare_dram_parameter("tokensT", [E, S], bf16, isOutput=False)
    d_wkv = nc.declare_dram_parameter("Wkv", [E, 128], bf16, isOutput=False)
    d_wq = nc.declare_dram_parameter("Wq", [E, H], bf16, isOutput=False)
    d_tm = nc.declare_dram_parameter("Tm", [6 * 128, TBL_W], bf16, isOutput=False)
    d_idf = nc.declare_dram_parameter("idf", [128, 128], f32, isOutput=False)
    d_out = nc.declare_dram_parameter("out", [1024, H], f32, isOutput=True)

    with ExitStack() as ctx:
        tc = ctx.enter_context(tile.TileContext(nc))
        const = ctx.enter_context(tc.tile_pool(name="const", bufs=1))
        pT_pool = ctx.enter_context(tc.tile_pool(name="pT", bufs=4))
        rec_pool = ctx.enter_context(tc.tile_pool(name="rec", bufs=4))
        avs_pool = ctx.enter_context(tc.tile_pool(name="avs", bufs=2))
        pp_pool = ctx.enter_context(tc.tile_pool(name="pp", bufs=2, space="PSUM"))
        ps_pool = ctx.enter_context(tc.tile_pool(name="ps", bufs=2, space="PSUM"))
        pav_pool = ctx.enter_context(tc.tile_pool(name="pav", bufs=2, space="PSUM"))
        ptr_pool = ctx.enter_context(tc.tile_pool(name="ptr", bufs=2, space="PSUM"))

        tokT = const.tile([128, 8 * S], bf16)       # e-tile et at cols et*S
        wkv = const.tile([128, 8 * 128], bf16)      # e-tile et at cols et*128
        wq = const.tile([128, 8 * H], bf16)
        tm = const.tile([128, 6 * TBL_W], bf16)
        idf = const.tile([128, 128], f32)
        idb = const.tile([128, 128], bf16)
        kvt = const.tile([128, S], bf16)            # rows 0:64 K^T, 64:128 V^T
        qt = const.tile([64, 1024], bf16)
        vn = const.tile([128, NT * 65], bf16)       # V natural + ones col per tile
        outs = const.tile([128, 512], f32)          # q-tile t at cols 64*t

        # ---- input DMAs ----
        nc.sync.dma_start(out=wkv[:].rearrange("p (e m) -> p e m", e=8),
                          in_=d_wkv[:].rearrange("(e p) m -> p e m", p=128))
        nc.sync.dma_start(out=wq[:].rearrange("p (e m) -> p e m", e=8),
                          in_=d_wq[:].rearrange("(e p) m -> p e m", p=128))
        nc.sync.dma_start(out=idf[:], in_=d_idf[:])
        tok_v = tokT[:].rearrange("p (e s) -> p e s", e=8)
        dtok_v = d_tok[:].rearrange("(e p) s -> p e s", p=128)
        for quarter in range(4):
            nc.sync.dma_start(out=tok_v[:, :, 512 * quarter:512 * (quarter + 1)],
                              in_=dtok_v[:, :, 512 * quarter:512 * (quarter + 1)])
        nc.scalar.dma_start(out=tm[:].rearrange("p (j g) -> p j g", j=6),
                            in_=d_tm[:].rearrange("(j p) g -> p j g", p=128))

        nc.vector.tensor_copy(idb[:], idf[:])   # f32 -> bf16 identity
        # ones columns of vn (col 64 of each 65-wide tile)
        nc.vector.memset(vn[:].rearrange("p (t c) -> p t c", c=65)[:, :, 64:65], 1.0)

        # ---- projections ----
        for c in range(4):
            pkv = pp_pool.tile([128, CH], f32, tag="pp")
            for e in range(8):
                nc.tensor.matmul(pkv[:], lhsT=wkv[:, 128 * e:128 * (e + 1)],
                                 rhs=tokT[:, S * e + CH * c: S * e + CH * (c + 1)],
                                 start=(e == 0), stop=(e == 7))
            nc.scalar.activation(kvt[0:64, CH * c:CH * (c + 1)], pkv[0:64, :],
                                 mybir.ActivationFunctionType.Copy)
            nc.vector.tensor_copy(kvt[64:128, CH * c:CH * (c + 1)], pkv[64:128, :])
            if c < 2:
                pq = pp_pool.tile([64, CH], f32, tag="pp")
                for e in range(8):
                    nc.tensor.matmul(pq[:], lhsT=wq[:, H * e:H * (e + 1)],
                                     rhs=tokT[:, S * e + CH * c: S * e + CH * (c + 1)],
                                     start=(e == 0), stop=(e == 7))
                nc.vector.tensor_copy(qt[:, CH * c:CH * (c + 1)], pq[:])
            # V natural tiles for this chunk (PE transpose of V^T)
            for t4 in range(4):
                t = 4 * c + t4
                ptr = ptr_pool.tile([128, 64], bf16, tag="ptr")
                nc.tensor.transpose(ptr[:], in_=kvt[64:128, 128 * t:128 * (t + 1)],
                                    identity=idb[64:128, 64:128])
                nc.vector.tensor_copy(vn[:, 65 * t:65 * t + 64], ptr[:])

        # ---- attention ----
        for ci, (phi, nslots, qoff) in enumerate([(PHI_A, SLOTS_A, 0),
                                                  (PHI_B, SLOTS_B, 512)]):
            pav = pav_pool.tile([65, CH], f32, tag="pav")
            for s in range(nslots):
                t = phi[s]
                psc = ps_pool.tile([128, CH], f32, tag="ps")
                nc.tensor.matmul(psc[:], lhsT=kvt[0:64, 128 * t:128 * (t + 1)],
                                 rhs=qt[:, qoff:qoff + CH], start=True, stop=True)
                pt = pT_pool.tile([128, CH], bf16, tag="pt")
                nc.scalar.activation(pt[:], psc[:], mybir.ActivationFunctionType.Exp,
                                     scale=float(E) ** -0.5)
                ptm = pT_pool.tile([128, CH], bf16, tag="ptm")
                j = (0 if ci == 0 else 2) + s // 4
                off = TBL_W * j + 384 - 128 * (s % 4)
                nc.vector.tensor_mul(ptm[:], pt[:], tm[:, off:off + CH])
                nc.tensor.matmul(pav[:], lhsT=vn[:, 65 * t:65 * (t + 1)], rhs=ptm[:],
                                 start=(s == 0), stop=(s == nslots - 1))
            avs = avs_pool.tile([65, CH], f32, tag="avs")
            nc.scalar.activation(avs[:], pav[:], mybir.ActivationFunctionType.Copy)
            for t4 in range(4):
                tro = ptr_pool.tile([128, 65], f32, tag="ptr")
                nc.tensor.transpose(tro[:], in_=avs[:, 128 * t4:128 * (t4 + 1)],
                                    identity=idf[0:65, 0:65])
                rec = rec_pool.tile([128, 1], f32, tag="rec")
                nc.vector.reciprocal(rec[:], tro[:, 64:65])
                col = 64 * (4 * ci + t4)
                nc.vector.tensor_scalar_mul(outs[:, col:col + 64], tro[:, 0:64], rec[:])
                nc.sync.dma_start(out=d_out[512 * ci + 128 * t4:512 * ci + 128 * (t4 + 1), :],
                                  in_=outs[:, col:col + 64])
    nc.finalize()
    return nc


def _get_compiled():
    global _COMPILED
    if _COMPILED is None:
        _COMPILED = _build()
    return _COMPILED


def kernel(tokens: np.ndarray, Wq: np.ndarray, Wk: np.ndarray, Wv: np.ndarray,
           _trace: bool = False, **_trace_kw):
    from concourse.bass_utils import run_bass_kernel_spmd

    nc = _get_compiled()
    wkv = np.concatenate([Wk, Wv], axis=1).astype(BF16)   # [1024, 128]
    wq = Wq.astype(BF16)
    idf = np.eye(128, dtype=np.float32)
    in_maps = []
    for i in range(NC):
        b, fold = i // 2, i % 2
        perm = _perm(fold)
        in_maps.append({
            "tokensT": np.ascontiguousarray(tokens[b][perm].T).astype(BF16),
            "Wkv": wkv,
            "Wq": wq,
            "Tm": _mask_tables(fold).reshape(6 * 128, TBL_W),
            "idf": idf,
        })
    res = run_bass_kernel_spmd(nc, in_maps, core_ids=list(range(NC)),
                               trace=_trace, **_trace_kw)
    out = np.empty((B, S, H), dtype=np.float32)
    for i in range(NC):
        b, fold = i // 2, i % 2
        perm = _perm(fold)
        out[b, perm[:1024]] = res.results[i]["out"]
    if _trace:
        return out, res
    return out
